# revision 1
# baseline (speedup 1.0000x reference)
"""Trainium2 Bass kernel for nn_DglHGTFFDConvBlock (HGT conv block).

Self-contained: host preprocessing (graph partitioning, weight folding),
bass/Tile kernel builder, SPMD runner over 8 NeuronCores, and kernel()
entry point taking full unsharded inputs and returning the full output.
"""
import base64  # noqa
import time
import numpy as np

# ---------------- walrus wait-count workaround ----------------

import bass_rust
import concourse.bass as bass


def _mk_wait_nop(nc, engine, waits):
    eng = nc.engines[engine]
    bi = eng.nop(hint="wait_spill", nofuse=True)
    inst = bi.ins
    # relocate: engine.nop appended it to cur_bb; pop it back out
    cur = nc.cur_bb
    lst = cur.bb.instructions if hasattr(cur, "bb") else cur.instructions
    popped = lst.pop()
    assert popped.name == inst.name, (popped.name, inst.name)
    inst.sync_info = bass_rust.SyncInfo(on_wait=list(waits), on_update=[])
    return inst


MAX_WAITS = 1


def legalize_waits(nc):
    n_spilled = 0
    for f in nc.m.functions:
        for bb in f.blocks:
            il = bb.instructions
            i = 0
            while i < len(il):
                inst = il[i]
                si = inst.sync_info
                if si is not None and si.on_wait and len(si.on_wait) > MAX_WAITS:
                    waits = list(si.on_wait)
                    si.on_wait = waits[:MAX_WAITS]
                    inst.sync_info = si
                    rest = waits[MAX_WAITS:]
                    for j in range(0, len(rest), MAX_WAITS):
                        wi = _mk_wait_nop(nc, inst.engine, rest[j:j + MAX_WAITS])
                        il.insert(i, wi)
                        i += 1
                        n_spilled += 1
                i += 1
    return n_spilled


# ---------------- SPMD runner ----------------

import time
import numpy as np
import jax
from jax.sharding import Mesh, PartitionSpec
from jax.experimental.shard_map import shard_map

import concourse.mybir as mybir
from concourse.bass2jax import (
    _bass_exec_p,
    install_neuronx_cc_hook,
    partition_id_tensor,
)


class SpmdRunner:
    def __init__(self, nc, n_cores=8):
        install_neuronx_cc_hook()
        assert nc.dbg_addr is None or not nc.dbg_callbacks
        self.nc = nc
        self.n_cores = n_cores
        partition_name = (
            nc.partition_id_tensor.name if nc.partition_id_tensor else None
        )
        in_names, out_names, out_avals, zero_outs = [], [], [], []
        for alloc in nc.m.functions[0].allocations:
            if not isinstance(alloc, mybir.MemoryLocationSet):
                continue
            name = alloc.memorylocations[0].name
            if alloc.kind == "ExternalInput":
                if name != partition_name:
                    in_names.append(name)
            elif alloc.kind == "ExternalOutput":
                out_names.append(name)
                shape = tuple(alloc.tensor_shape)
                dtype = mybir.dt.np(alloc.dtype)
                out_avals.append(jax.core.ShapedArray(shape, dtype))
                zero_outs.append(np.zeros(shape, dtype))
        self.in_names, self.out_names = in_names, out_names
        self.zero_outs = zero_outs
        n_params, n_outs = len(in_names), len(out_names)
        self.n_params = n_params
        all_in_names = list(in_names) + list(out_names)
        if partition_name is not None:
            all_in_names.append(partition_name)

        def _body(*args):
            operands = list(args)
            if partition_name is not None:
                operands.append(partition_id_tensor())
            outs = _bass_exec_p.bind(
                *operands,
                out_avals=tuple(out_avals),
                in_names=tuple(all_in_names),
                out_names=tuple(out_names),
                lowering_input_output_aliases=(),
                sim_require_finite=True,
                sim_require_nnan=True,
                nc=nc,
            )
            return tuple(outs)

        devices = jax.devices()[:n_cores]
        self.mesh = Mesh(np.asarray(devices), ("core",))
        in_specs = (PartitionSpec("core"),) * (n_params + n_outs)
        out_specs = (PartitionSpec("core"),) * n_outs
        # NOTE: no donation — lets us re-run on the same on-device inputs.
        # Outputs are fully written by our kernels, so no zero-init needed.
        self.fn = jax.jit(
            shard_map(_body, mesh=self.mesh, in_specs=in_specs,
                      out_specs=out_specs, check_rep=False),
            keep_unused=True,
        )

    def put_inputs(self, in_maps):
        """in_maps: list (per core) of dicts name->np array."""
        concat = [
            np.concatenate([np.asarray(in_maps[c][n]) for n in [name]
                            for c in range(self.n_cores)], axis=0)
            for name in self.in_names
        ]
        concat += [
            np.concatenate([z for _ in range(self.n_cores)], axis=0)
            if False else np.concatenate([z] * self.n_cores, axis=0)
            for z in self.zero_outs
        ]
        sharding = jax.sharding.NamedSharding(self.mesh, PartitionSpec("core"))
        self.dev_args = [jax.device_put(a, sharding) for a in concat]
        jax.block_until_ready(self.dev_args)

    def run(self):
        outs = self.fn(*self.dev_args)
        jax.block_until_ready(outs)
        return outs

    def time_runs(self, iters=10, warmup=2):
        for _ in range(warmup):
            self.run()
        times = []
        for _ in range(iters):
            t0 = time.perf_counter()
            self.run()
            times.append(time.perf_counter() - t0)
        return min(times), times

    def results(self, outs=None):
        if outs is None:
            outs = self.run()
        res = []
        for c in range(self.n_cores):
            d = {}
            for i, name in enumerate(self.out_names):
                arr = np.asarray(outs[i])
                per = arr.shape[0] // self.n_cores
                d[name] = arr[c * per:(c + 1) * per]
            res.append(d)
        return res


# ---------------- HGT kernel ----------------

import numpy as np

import concourse.bass as bass
import concourse.mybir as mybir
from concourse.tile import TileContext
from concourse.masks import make_identity

F32 = mybir.dt.float32
I32 = mybir.dt.int32
AX = mybir.AxisListType
OP = mybir.AluOpType
ACTF = mybir.ActivationFunctionType

H = 8
C = 8

REL_SRC = [0, 1, 0]
REL_DST = [1, 0, 0]
TYPE_RELS = {0: [1, 2], 1: [0]}

CHUNK = 2
NEG_BIG = -1e30
LN_EPS = 1e-5


class Cfg:
    def __init__(self, N=100000, E=500000, IN=128, OUT=128, DFF=512):
        self.N, self.E, self.IN, self.OUT, self.DFF = N, E, IN, OUT, DFF
        self.DK = OUT // H
        self.S = N // C
        self.NT = (self.S + 127) // 128
        self.SPAD = self.NT * 128


def blockdiag(mats):
    h, dk = mats.shape[0], mats.shape[1]
    bd = np.zeros((h * dk, h * dk), np.float32)
    for i in range(h):
        bd[i * dk:(i + 1) * dk, i * dk:(i + 1) * dk] = mats[i]
    return bd


def preprocess(inputs, cfg):
    N, S, NT, SPAD = cfg.N, cfg.S, cfg.NT, cfg.SPAD
    h_a = np.ascontiguousarray(np.asarray(inputs['h_a'], np.float32))
    h_b = np.ascontiguousarray(np.asarray(inputs['h_b'], np.float32))
    hcat = np.concatenate([h_a, h_b], axis=0)

    rel_att = np.asarray(inputs['rel_att'], np.float32)
    rel_msg = np.asarray(inputs['rel_msg'], np.float32)
    rel_pri = np.asarray(inputs['rel_pri'], np.float32)
    Wk = np.asarray(inputs['Wk'], np.float32)
    Wq = np.asarray(inputs['Wq'], np.float32)
    Wv = np.asarray(inputs['Wv'], np.float32)
    bk = np.asarray(inputs['bk'], np.float32)
    bq = np.asarray(inputs['bq'], np.float32)
    bv = np.asarray(inputs['bv'], np.float32)

    Wq_f, bq_f, Wkv_f, bkv_f = [], [], [], []
    for r in range(3):
        ts, td = REL_SRC[r], REL_DST[r]
        scale = np.repeat(rel_pri[r] / np.sqrt(cfg.DK), cfg.DK)
        Wq_f.append(Wq[td] * scale[None, :])
        bq_f.append(bq[td] * scale)
        wkp = Wk[ts] @ blockdiag(rel_att[r])
        wvp = Wv[ts] @ blockdiag(rel_msg[r])
        Wkv_f.append(np.concatenate([wkp, wvp], axis=1))
        bkv_f.append(np.concatenate([bk[ts] @ blockdiag(rel_att[r]),
                                     bv[ts] @ blockdiag(rel_msg[r])]))
    Wq_f, bq_f = np.stack(Wq_f), np.stack(bq_f)
    Wkv_f, bkv_f = np.stack(Wkv_f), np.stack(bkv_f)

    gamma = np.asarray(inputs['gamma'], np.float32)
    beta = np.asarray(inputs['beta'], np.float32)
    W1 = np.asarray(inputs['W1'], np.float32)
    b1 = np.asarray(inputs['b1'], np.float32)
    Wa = np.asarray(inputs['Wa'], np.float32)
    ba = np.asarray(inputs['ba'], np.float32)
    W2 = np.asarray(inputs['W2'], np.float32)
    b2 = np.asarray(inputs['b2'], np.float32)
    W1g = gamma[:, :, None] * W1
    c1 = np.einsum('tf,tfd->td', beta, W1) + b1

    weights = dict(Wq_f=Wq_f, bq_f=bq_f, Wkv_f=Wkv_f, bkv_f=bkv_f,
                   Wa=np.ascontiguousarray(Wa), ba=np.ascontiguousarray(ba),
                   W1g=np.ascontiguousarray(W1g), c1=np.ascontiguousarray(c1),
                   W2=np.ascontiguousarray(W2), b2=np.ascontiguousarray(b2))
    use_bias_qkv = bool(np.abs(bq_f).max() > 0 or np.abs(bkv_f).max() > 0)
    use_bias_ffn = bool(np.abs(ba).max() > 0 or np.abs(c1).max() > 0
                        or np.abs(b2).max() > 0)

    per_core = [dict() for _ in range(C)]
    Ds = []
    for r in range(3):
        src = np.asarray(inputs[f'src{r}']).astype(np.int64)
        dst = np.asarray(inputs[f'dst{r}']).astype(np.int64)
        ts, td = REL_SRC[r], REL_DST[r]
        core_data = []
        for c in range(C):
            lo = c * S
            m = (dst >= lo) & (dst < lo + S)
            ds = dst[m] - lo
            ss = src[m]
            o = np.argsort(ds, kind='stable')
            ds, ss = ds[o], ss[o]
            deg = np.bincount(ds, minlength=S)
            starts = np.concatenate([[0], np.cumsum(deg)])
            order = np.argsort(-deg, kind='stable')
            order_pad = np.concatenate([order, np.zeros(SPAD - S, np.int64)])
            deg_pad = np.concatenate([deg[order], np.zeros(SPAD - S, np.int64)])
            core_data.append((order_pad, deg_pad, ss, starts, ds))
        D_r = [max(int(cd[1][t * 128:(t + 1) * 128].max())
                   for cd in core_data) for t in range(NT)]
        # NOTE: addr layout needs per-tile flat layout [128, D] row-major
        Ds.append(D_r)
        D_arr = np.asarray(D_r, np.int64)
        tile_base = np.concatenate([[0], np.cumsum(128 * D_arr)])
        total = int(tile_base[-1])
        for c in range(C):
            order_pad, deg_pad, ss_c, starts, ds_c = core_data[c]
            pos = np.zeros(SPAD, np.int32)
            pos[order_pad[:S]] = np.arange(S, dtype=np.int32)
            # vectorized edge placement: edge (dst n, rank k) -> slot addr
            n_e = len(ss_c)
            if n_e:
                p0 = pos[ds_c].astype(np.int64)        # sorted position of dst
                tt = p0 >> 7
                row = p0 & 127
                rank = np.arange(n_e, dtype=np.int64) - starts[ds_c]
                addr = tile_base[tt] + row * D_arr[tt] + rank
                idx_full = np.zeros(max(total, 1), np.int32)
                msk_full = np.zeros(max(total, 1), np.float32)
                idx_full[addr] = (ss_c + ts * N).astype(np.int32)
                msk_full[addr] = 1.0
            else:
                idx_full = np.zeros(1, np.int32)
                msk_full = np.zeros(1, np.float32)
            per_core[c][f'idx{r}'] = idx_full
            per_core[c][f'mask{r}'] = msk_full
            per_core[c][f'pos{r}'] = pos
            hsrt = hcat[(order_pad + c * S + td * N).astype(np.int64)]
            per_core[c][f'hsort{r}'] = np.ascontiguousarray(hsrt)

    schedule = dict(Ds=Ds, use_bias_qkv=use_bias_qkv, use_bias_ffn=use_bias_ffn)
    in_maps = []
    for c in range(C):
        m = dict(per_core[c])
        m['hcat'] = hcat
        # own residual rows, natural order, padded: [2*SPAD, IN]
        hown = np.zeros((2 * SPAD, cfg.IN), np.float32)
        hown[:S] = h_a[c * S:(c + 1) * S]
        hown[SPAD:SPAD + S] = h_b[c * S:(c + 1) * S]
        m['hown'] = hown
        m.update(weights)
        in_maps.append(m)
    return schedule, in_maps


def schedule_key(schedule, cfg, reps):
    return (cfg.N, cfg.E, reps, tuple(tuple(d) for d in schedule['Ds']),
            schedule['use_bias_qkv'], schedule['use_bias_ffn'])


def build(schedule, cfg, reps=1, debug_tile=None):
    N, S, NT, SPAD = cfg.N, cfg.S, cfg.NT, cfg.SPAD
    OUTF, DFF = cfg.OUT, cfg.DFF
    Ds = schedule['Ds']
    bias_qkv = schedule['use_bias_qkv']
    bias_ffn = schedule['use_bias_ffn']
    DMAX = max(max(d) for d in Ds)

    nc = bass.Bass(dynamic_dma_scratch_size=2**16)
    P = {}
    P['hcat'] = nc.declare_dram_parameter('hcat', [2 * N, cfg.IN], F32,
                                          isOutput=False)
    P['hown'] = nc.declare_dram_parameter('hown', [2 * SPAD, cfg.IN], F32,
                                          isOutput=False)
    for r in range(3):
        tot = max(sum(128 * D for D in Ds[r]), 1)
        totm = max(sum(128 * D for D in Ds[r]), 1)
        P[f'idx{r}'] = nc.declare_dram_parameter(f'idx{r}', [tot], I32, isOutput=False)
        P[f'hsort{r}'] = nc.declare_dram_parameter(f'hsort{r}', [SPAD, cfg.IN],
                                                   F32, isOutput=False)
        P[f'mask{r}'] = nc.declare_dram_parameter(f'mask{r}', [totm], F32, isOutput=False)
        P[f'pos{r}'] = nc.declare_dram_parameter(f'pos{r}', [SPAD], I32, isOutput=False)
    P['Wq_f'] = nc.declare_dram_parameter('Wq_f', [3, 128, 128], F32, isOutput=False)
    P['bq_f'] = nc.declare_dram_parameter('bq_f', [3, 128], F32, isOutput=False)
    P['Wkv_f'] = nc.declare_dram_parameter('Wkv_f', [3, 128, 256], F32, isOutput=False)
    P['bkv_f'] = nc.declare_dram_parameter('bkv_f', [3, 256], F32, isOutput=False)
    P['Wa'] = nc.declare_dram_parameter('Wa', [2, 128, 128], F32, isOutput=False)
    P['ba'] = nc.declare_dram_parameter('ba', [2, 128], F32, isOutput=False)
    P['W1g'] = nc.declare_dram_parameter('W1g', [2, 128, DFF], F32, isOutput=False)
    P['c1'] = nc.declare_dram_parameter('c1', [2, DFF], F32, isOutput=False)
    P['W2'] = nc.declare_dram_parameter('W2', [2, DFF, 128], F32, isOutput=False)
    P['b2'] = nc.declare_dram_parameter('b2', [2, 128], F32, isOutput=False)
    out_sb = [nc.declare_dram_parameter('out_a', [SPAD, OUTF], F32, isOutput=True),
              nc.declare_dram_parameter('out_b', [SPAD, OUTF], F32, isOutput=True)]
    tR = [nc.dram_tensor(f'tR{r}', [SPAD, OUTF], F32) for r in range(3)]

    with TileContext(nc) as tc:
        with tc.tile_pool(name="const", bufs=1) as cp, \
             tc.tile_pool(name="work", bufs=3) as wp, \
             tc.tile_pool(name="big", bufs=2) as bp, \
             tc.tile_pool(name="gat", bufs=40) as gp, \
             tc.tile_pool(name="ps", bufs=2, space="PSUM") as pp, \
             tc.tile_pool(name="pkv", bufs=2, space="PSUM") as pkv:

            ident = cp.tile([128, 128], F32)
            make_identity(nc, ident[:])
            ones1 = cp.tile([1, 128], F32)
            nc.vector.memset(ones1[:], 1.0)

            wq_sb = cp.tile([128, 3 * 128], F32)
            nc.sync.dma_start(out=wq_sb[:].rearrange("p (r f) -> p r f", f=128),
                              in_=P['Wq_f'][:].rearrange("r k f -> k r f"))
            wkv_sb = cp.tile([128, 3 * 256], F32)
            nc.sync.dma_start(out=wkv_sb[:].rearrange("p (r f) -> p r f", f=256),
                              in_=P['Wkv_f'][:].rearrange("r k f -> k r f"))
            wa_sb = cp.tile([128, 2 * 128], F32)
            nc.sync.dma_start(out=wa_sb[:].rearrange("p (t f) -> p t f", f=128),
                              in_=P['Wa'][:].rearrange("t k f -> k t f"))
            w1_sb = cp.tile([128, 2 * DFF], F32)
            nc.sync.dma_start(out=w1_sb[:].rearrange("p (t f) -> p t f", f=DFF),
                              in_=P['W1g'][:].rearrange("t k f -> k t f"))
            w2_sb = cp.tile([128, 2 * 4 * 128], F32)
            nc.sync.dma_start(
                out=w2_sb[:].rearrange("p (t c f) -> p t c f", c=4, f=128),
                in_=P['W2'][:].rearrange("t (c k) f -> k t c f", k=128))
            if bias_qkv:
                bq_sb = cp.tile([1, 3 * 128], F32)
                nc.sync.dma_start(out=bq_sb[:],
                                  in_=P['bq_f'][:].rearrange("r f -> 1 (r f)"))
                bkv_sb = cp.tile([1, 3 * 256], F32)
                nc.sync.dma_start(out=bkv_sb[:],
                                  in_=P['bkv_f'][:].rearrange("r f -> 1 (r f)"))
            if bias_ffn:
                ba_sb = cp.tile([1, 2 * 128], F32)
                nc.sync.dma_start(out=ba_sb[:],
                                  in_=P['ba'][:].rearrange("t f -> 1 (t f)"))
                c1_sb = cp.tile([1, 2 * DFF], F32)
                nc.sync.dma_start(out=c1_sb[:],
                                  in_=P['c1'][:].rearrange("t f -> 1 (t f)"))
                b2_sb = cp.tile([1, 2 * 128], F32)
                nc.sync.dma_start(out=b2_sb[:],
                                  in_=P['b2'][:].rearrange("t f -> 1 (t f)"))

            dbg = {}
            def dump(name, ap, shape):
                t = nc.declare_dram_parameter('dbg_' + name, list(shape), F32,
                                              isOutput=True)
                nc.sync.dma_start(out=t[:], in_=ap)
                dbg[name] = t

            def ap3(tile_ap, offset_add, dims):
                """Manual AP: partition dim of tile_ap + given [stride,count] dims."""
                return bass.AP(tile_ap.tensor, tile_ap.offset + offset_add,
                               [tile_ap.ap[0]] + dims)

            def pass_a(r):
                idx_off, mask_off = 0, 0
                for t in range(NT):
                    D = Ds[r][t]
                    if D == 0:
                        zt = wp.tile([128, OUTF], F32, tag="ttile")
                        nc.vector.memset(zt[:], 0.0)
                        nc.sync.dma_start(out=tR[r][t * 128:(t + 1) * 128, :],
                                          in_=zt[:])
                        continue
                    idx_sb = wp.tile([128, D], I32, tag="idx")
                    nc.sync.dma_start(
                        out=idx_sb[:],
                        in_=P[f'idx{r}'][idx_off:idx_off + 128 * D]
                            .rearrange("(p d) -> p d", d=D))
                    idx_off += 128 * D
                    mask_sb = wp.tile([128, D], F32, tag="mask")
                    nc.sync.dma_start(
                        out=mask_sb[:],
                        in_=P[f'mask{r}'][mask_off:mask_off + 128 * D]
                            .rearrange("(p d) -> p d", d=D))
                    mask_off += 128 * D

                    hs = wp.tile([128, 128], F32, tag="hs")
                    nc.sync.dma_start(
                        out=hs[:],
                        in_=P[f'hsort{r}'][t * 128:(t + 1) * 128, :])
                    hg_slots = []
                    for j in range(D):
                        gj = gp.tile([128, 128], F32, tag="hgs")
                        nc.gpsimd.indirect_dma_start(
                            out=gj[:], out_offset=None,
                            in_=P['hcat'][:, :],
                            in_offset=bass.IndirectOffsetOnAxis(
                                ap=idx_sb[:, j:j + 1], axis=0))
                        hg_slots.append(gj)

                    tp = pp.tile([128, 128], F32, space="PSUM", tag="tp")
                    nc.tensor.transpose(out=tp[:], in_=hs[:],
                                        identity=ident[:])
                    hdT = wp.tile([128, 128], F32, tag="hdT")
                    nc.scalar.activation(out=hdT[:], in_=tp[:], func=ACTF.Copy)
                    qp = pp.tile([128, 128], F32, space="PSUM", tag="tp")
                    nc.tensor.matmul(out=qp[:], lhsT=hdT[:],
                                     rhs=wq_sb[:, r * 128:(r + 1) * 128],
                                     start=True, stop=not bias_qkv)
                    if bias_qkv:
                        nc.tensor.matmul(out=qp[:], lhsT=ones1[:],
                                         rhs=bq_sb[:, r * 128:(r + 1) * 128],
                                         start=False, stop=True)
                    q_sb = wp.tile([128, 128], F32, tag="q")
                    nc.scalar.activation(out=q_sb[:], in_=qp[:], func=ACTF.Copy)

                    s_sb = wp.tile([128, H * DMAX], F32, tag="s")
                    vps = bp.tile([128, DMAX * 128], F32, tag="vps")
                    dbg_on = debug_tile == (r, t)
                    if dbg_on:
                        dump('q', q_sb[:], (128, 128))

                    for j0 in range(0, D, CHUNK):
                        cn = min(CHUNK, D - j0)
                        tpc = pkv.tile([128, CHUNK * 128], F32,
                                       space="PSUM", tag="tpc")
                        for j in range(cn):
                            nc.tensor.transpose(
                                out=tpc[:, j * 128:(j + 1) * 128],
                                in_=hg_slots[j0 + j][:],
                                identity=ident[:])
                        hgT = wp.tile([128, CHUNK * 128], F32, tag="hgT")
                        nc.scalar.activation(out=hgT[:, :cn * 128],
                                             in_=tpc[:, :cn * 128], func=ACTF.Copy)
                        kv = pkv.tile([128, CHUNK * 256], F32,
                                      space="PSUM", tag="kv")
                        for j in range(cn):
                            nc.tensor.matmul(
                                out=kv[:, j * 256:(j + 1) * 256],
                                lhsT=hgT[:, j * 128:(j + 1) * 128],
                                rhs=wkv_sb[:, r * 256:(r + 1) * 256],
                                start=True, stop=not bias_qkv)
                            if bias_qkv:
                                nc.tensor.matmul(
                                    out=kv[:, j * 256:(j + 1) * 256],
                                    lhsT=ones1[:],
                                    rhs=bkv_sb[:, r * 256:(r + 1) * 256],
                                    start=False, stop=True)
                        prod = wp.tile([128, CHUNK * 128], F32, tag="prod")
                        kp_ap = ap3(kv[:], 0, [[256, cn], [1, 128]])
                        q_bc = ap3(q_sb[:], 0, [[0, cn], [1, 128]])
                        nc.vector.tensor_tensor(
                            out=prod[:, :cn * 128].rearrange(
                                "p (c f) -> p c f", f=128),
                            in0=kp_ap, in1=q_bc, op=OP.mult)
                        s_out = ap3(s_sb[:], j0, [[1, cn], [DMAX, H]])
                        nc.vector.tensor_reduce(
                            out=s_out,
                            in_=prod[:, :cn * 128].rearrange(
                                "p (c h d) -> p c h d", h=H, d=cfg.DK),
                            axis=AX.X, op=OP.add)
                        vp_ap = ap3(kv[:], 128, [[256, cn], [1, 128]])
                        nc.scalar.activation(
                            out=vps[:, j0 * 128:(j0 + cn) * 128].rearrange(
                                "p (c f) -> p c f", f=128),
                            in_=vp_ap, func=ACTF.Copy)

                    if dbg_on:
                        dump('s_raw', s_sb[:], (128, H * DMAX))
                        dump('vps', vps[:, :D * 128], (128, D * 128))
                    # masked softmax, head-major s[h*DMAX + j]
                    mb = wp.tile([128, DMAX], F32, tag="mb")
                    nc.vector.tensor_scalar(
                        out=mb[:, :D], in0=mask_sb[:, :D],
                        scalar1=-NEG_BIG, scalar2=NEG_BIG,
                        op0=OP.mult, op1=OP.add)
                    s3 = ap3(s_sb[:], 0, [[DMAX, H], [1, D]])
                    mb3 = ap3(mb[:], 0, [[0, H], [1, D]])
                    nc.vector.tensor_tensor(out=s3, in0=s3, in1=mb3, op=OP.add)
                    negm = wp.tile([128, H], F32, tag="negm")
                    nc.vector.tensor_reduce(out=negm[:], in_=s3, axis=AX.X,
                                            op=OP.max, negate=True)
                    negm3 = ap3(negm[:], 0, [[1, H], [0, D]])
                    nc.vector.tensor_tensor(out=s3, in0=s3, in1=negm3, op=OP.add)
                    if dbg_on:
                        dump('s_shift', s_sb[:], (128, H * DMAX))
                        dump('negm', negm[:], (128, H))
                    e_sb = wp.tile([128, H * DMAX], F32, tag="e")
                    e3 = ap3(e_sb[:], 0, [[DMAX, H], [1, D]])
                    nc.scalar.activation(out=e3, in_=s3, func=ACTF.Exp)
                    z_sb = wp.tile([128, H], F32, tag="z")
                    nc.vector.tensor_reduce(out=z_sb[:], in_=e3, axis=AX.X,
                                            op=OP.add)
                    rz = wp.tile([128, H], F32, tag="rz")
                    nc.vector.reciprocal(rz[:], z_sb[:])
                    rz3 = ap3(rz[:], 0, [[1, H], [0, D]])
                    a_sb = wp.tile([128, H * DMAX], F32, tag="a")
                    a3 = ap3(a_sb[:], 0, [[DMAX, H], [1, D]])
                    nc.vector.tensor_tensor(out=a3, in0=e3, in1=rz3, op=OP.mult)
                    mask3 = ap3(mask_sb[:], 0, [[0, H], [1, D]])
                    nc.vector.tensor_tensor(out=a3, in0=a3, in1=mask3, op=OP.mult)

                    if dbg_on:
                        dump('e', e_sb[:], (128, H * DMAX))
                        dump('z', z_sb[:], (128, H))
                        dump('a', a_sb[:], (128, H * DMAX))
                    a_agg = ap3(a_sb[:], 0, [[1, D], [DMAX, H], [0, cfg.DK]])
                    nc.vector.tensor_tensor(
                        out=vps[:, :D * 128].rearrange(
                            "p (c h d) -> p c h d", h=H, d=cfg.DK),
                        in0=vps[:, :D * 128].rearrange(
                            "p (c h d) -> p c h d", h=H, d=cfg.DK),
                        in1=a_agg, op=OP.mult)
                    ttile = wp.tile([128, OUTF], F32, tag="ttile")
                    tmp_red = ap3(vps[:], 0, [[1, 128], [128, D]])
                    nc.vector.tensor_reduce(out=ttile[:], in_=tmp_red,
                                            axis=AX.X, op=OP.add)
                    nc.sync.dma_start(out=tR[r][t * 128:(t + 1) * 128, :],
                                      in_=ttile[:])

            def pass_b(ty):
                if True:
                    rels = TYPE_RELS[ty]
                    for t in range(NT):
                        row0 = t * 128
                        tg = []
                        for r in rels:
                            pos_sb = wp.tile([128, 1], I32, tag="pos")
                            nc.sync.dma_start(
                                out=pos_sb[:],
                                in_=P[f'pos{r}'][row0:row0 + 128]
                                    .rearrange("(p one) -> p one", one=1))
                            g = wp.tile([128, OUTF], F32, tag=f"tg{r}")
                            nc.gpsimd.indirect_dma_start(
                                out=g[:], out_offset=None, in_=tR[r][:, :],
                                in_offset=bass.IndirectOffsetOnAxis(
                                    ap=pos_sb[:, :1], axis=0))
                            tg.append(g)
                        if len(tg) == 2:
                            x0 = wp.tile([128, OUTF], F32, tag="x0")
                            nc.vector.tensor_tensor(out=x0[:], in0=tg[0][:],
                                                    in1=tg[1][:], op=OP.add)
                        else:
                            x0 = tg[0]
                        tp = pp.tile([128, 128], F32, space="PSUM", tag="tp")
                        nc.tensor.transpose(out=tp[:], in_=x0[:], identity=ident[:])
                        rT = wp.tile([128, 128], F32, tag="rT")
                        nc.scalar.activation(out=rT[:], in_=tp[:], func=ACTF.Relu)
                        y1 = pp.tile([128, 128], F32, space="PSUM", tag="qp")
                        nc.tensor.matmul(out=y1[:], lhsT=rT[:],
                                         rhs=wa_sb[:, ty * 128:(ty + 1) * 128],
                                         start=True, stop=not bias_ffn)
                        if bias_ffn:
                            nc.tensor.matmul(out=y1[:], lhsT=ones1[:],
                                             rhs=ba_sb[:, ty * 128:(ty + 1) * 128],
                                             start=False, stop=True)
                        h_sb = wp.tile([128, 128], F32, tag="hres")
                        nc.sync.dma_start(
                            out=h_sb[:],
                            in_=P['hown'][ty * SPAD + row0:ty * SPAD + row0 + 128, :])
                        x_sb = wp.tile([128, 128], F32, tag="x")
                        nc.vector.tensor_tensor(out=x_sb[:], in0=y1[:],
                                                in1=h_sb[:], op=OP.add)
                        nmu = wp.tile([128, 1], F32, tag="nmu")
                        nc.vector.tensor_reduce(out=nmu[:], in_=x_sb[:],
                                                axis=AX.X, op=OP.add, negate=True)
                        nc.vector.tensor_scalar_mul(nmu[:], nmu[:], 1.0 / 128)
                        xc = wp.tile([128, 128], F32, tag="xc")
                        nc.vector.tensor_scalar_add(xc[:], x_sb[:], nmu[:, :1])
                        sq = wp.tile([128, 128], F32, tag="sq")
                        ssq = wp.tile([128, 1], F32, tag="ssq")
                        nc.scalar.activation(out=sq[:], in_=xc[:], func=ACTF.Square,
                                             accum_out=ssq[:, :1])
                        std = wp.tile([128, 1], F32, tag="std")
                        nc.vector.tensor_scalar(out=std[:], in0=ssq[:],
                                                scalar1=1.0 / 128, scalar2=LN_EPS,
                                                op0=OP.mult, op1=OP.add)
                        nc.scalar.activation(out=std[:], in_=std[:], func=ACTF.Sqrt)
                        rstd = wp.tile([128, 1], F32, tag="rstd")
                        nc.vector.reciprocal(rstd[:], std[:])
                        xn = wp.tile([128, 128], F32, tag="xn")
                        nc.vector.tensor_scalar_mul(xn[:], xc[:], rstd[:, :1])
                        tp2 = pp.tile([128, 128], F32, space="PSUM", tag="tp")
                        nc.tensor.transpose(out=tp2[:], in_=xn[:], identity=ident[:])
                        xnT = wp.tile([128, 128], F32, tag="xnT")
                        nc.scalar.activation(out=xnT[:], in_=tp2[:], func=ACTF.Copy)
                        y2 = pkv.tile([128, DFF], F32, space="PSUM", tag="kv")
                        nc.tensor.matmul(out=y2[:], lhsT=xnT[:],
                                         rhs=w1_sb[:, ty * DFF:(ty + 1) * DFF],
                                         start=True, stop=not bias_ffn)
                        if bias_ffn:
                            nc.tensor.matmul(out=y2[:], lhsT=ones1[:],
                                             rhs=c1_sb[:, ty * DFF:(ty + 1) * DFF],
                                             start=False, stop=True)
                        r2 = wp.tile([128, DFF], F32, tag="r2")
                        nc.scalar.activation(out=r2[:], in_=y2[:], func=ACTF.Relu)
                        yo = pp.tile([128, 128], F32, space="PSUM", tag="qp")
                        nch = DFF // 128
                        for cidx in range(nch):
                            tp3 = pp.tile([128, 128], F32, space="PSUM", tag="tp")
                            nc.tensor.transpose(
                                out=tp3[:], in_=r2[:, cidx * 128:(cidx + 1) * 128],
                                identity=ident[:])
                            r2T = wp.tile([128, 128], F32, tag="r2T")
                            nc.scalar.activation(out=r2T[:], in_=tp3[:],
                                                 func=ACTF.Copy)
                            nc.tensor.matmul(
                                out=yo[:], lhsT=r2T[:],
                                rhs=w2_sb[:, (ty * nch + cidx) * 128:
                                          (ty * nch + cidx + 1) * 128],
                                start=(cidx == 0),
                                stop=(cidx == nch - 1) and not bias_ffn)
                        if bias_ffn:
                            nc.tensor.matmul(out=yo[:], lhsT=ones1[:],
                                             rhs=b2_sb[:, ty * 128:(ty + 1) * 128],
                                             start=False, stop=True)
                        o_sb = wp.tile([128, 128], F32, tag="o")
                        nc.scalar.activation(out=o_sb[:], in_=yo[:], func=ACTF.Copy)
                        nc.sync.dma_start(out=out_sb[ty][row0:row0 + 128, :],
                                          in_=o_sb[:])

            def body():
                pass_a(0)
                pass_b(1)
                pass_a(1)
                pass_a(2)
                pass_b(0)

            if reps == 1:
                body()
            else:
                with tc.For_i(0, reps, 1):
                    body()

    legalize_waits(nc)
    return nc


def assemble_output(results, cfg):
    out = np.empty((2, cfg.N, cfg.OUT), np.float32)
    for c in range(C):
        out[0, c * cfg.S:(c + 1) * cfg.S] = results[c]['out_a'][:cfg.S]
        out[1, c * cfg.S:(c + 1) * cfg.S] = results[c]['out_b'][:cfg.S]
    return out


def numpy_device_sim(schedule, in_map, cfg):
    """Mirror the device algorithm in numpy for one core."""
    Ds = schedule['Ds']
    hcat = in_map['hcat']
    S, NT, SPAD = cfg.S, cfg.NT, cfg.SPAD
    tRs = []
    for r in range(3):
        Wq_f = in_map['Wq_f'][r]; bq_f = in_map['bq_f'][r]
        Wkv = in_map['Wkv_f'][r]; bkv = in_map['bkv_f'][r]
        idx_off, mask_off = 0, 0
        tr = np.zeros((SPAD, cfg.OUT), np.float32)
        for t in range(NT):
            D = Ds[r][t]
            if D == 0:
                continue
            idx = in_map[f'idx{r}'][idx_off:idx_off + 128 * D]\
                .reshape(128, D)
            idx_off += 128 * D
            msk = in_map[f'mask{r}'][mask_off:mask_off + 128 * D]\
                .reshape(128, D)
            mask_off += 128 * D
            hs = in_map[f'hsort{r}'][t * 128:(t + 1) * 128]
            hg = hcat[idx]
            q = hs @ Wq_f + bq_f
            kv = hg @ Wkv + bkv
            kp, vp = kv[..., :128], kv[..., 128:]
            s = (kp.reshape(128, D, H, cfg.DK)
                 * q.reshape(128, 1, H, cfg.DK)).sum(-1)     # [128, D, H]
            s = s + (msk * 1e30 - 1e30)[:, :, None]
            m = s.max(axis=1, keepdims=True)
            e = np.exp(s - m)
            z = e.sum(axis=1, keepdims=True)
            a = e / z * msk[:, :, None]
            tt = (vp.reshape(128, D, H, cfg.DK) * a[..., None]).sum(1)
            tr[t * 128:(t + 1) * 128] = tt.reshape(128, cfg.OUT)
        tRs.append(tr)
    outs = []
    for ty in (0, 1):
        res = np.zeros((SPAD, cfg.OUT), np.float32)
        for t in range(NT):
            row0 = t * 128
            x0 = np.zeros((128, cfg.OUT), np.float32)
            for r in TYPE_RELS[ty]:
                pos = in_map[f'pos{r}'][row0:row0 + 128]
                x0 = x0 + tRs[r][pos]
            x = np.maximum(x0, 0) @ in_map['Wa'][ty] + in_map['ba'][ty]
            x = x + in_map['hown'][ty * SPAD + row0: ty * SPAD + row0 + 128]
            mu = x.mean(-1, keepdims=True)
            var = ((x - mu) ** 2).mean(-1, keepdims=True)
            xn = (x - mu) / np.sqrt(var + LN_EPS)
            y = np.maximum(xn @ in_map['W1g'][ty] + in_map['c1'][ty], 0)
            res[row0:row0 + 128] = y @ in_map['W2'][ty] + in_map['b2'][ty]
        outs.append(res)
    return outs[0], outs[1], tRs


# ---------------- entry point ----------------
_CACHE = {}


def kernel(**inputs):
    cfg = Cfg()
    inputs = {k: np.asarray(v) for k, v in inputs.items()}
    schedule, in_maps = preprocess(inputs, cfg)
    key = schedule_key(schedule, cfg, 1)
    if key not in _CACHE:
        nc = build(schedule, cfg, reps=1)
        _CACHE[key] = SpmdRunner(nc, 8)
    runner = _CACHE[key]
    runner.put_inputs(in_maps)
    outs = runner.run()
    res = runner.results(outs)
    return assemble_output(res, cfg)

